# revision 52
# baseline (speedup 1.0000x reference)
"""Multi-head attention (B=4, H=16, S=2048, D=128, causal+pad mask) on 8 TRN2 NeuronCores.

Sharding: the 64 (batch, head) pairs are split 8 per core (pure data parallel —
attention is independent per head, no collectives needed).

Per-core kernel (per head):
  - scores are computed TRANSPOSED: S^T[k, q] = K_block^T^T @ Q^T with the
    contraction dim d=128 on partitions, the k-block (128) as the PSUM partition
    dim and the allowed q-columns (128-chunk granular, derived from the actual
    mask) as the moving dim. Q/K are host-cast to bf16.
  - The allowed 128x128 chunks are packed into PSUM group tiles of up to 12
    chunks ([128, 1536] f32 = 3 banks) so ONE scalar-engine ACTIVATE computes
    exp(scale*s) for the whole group out of PSUM into SBUF bf16 — the ACT
    engine is the critical path (1 col/cycle @1.2GHz), so group count is
    minimized. Bins never span more than 2 q-megatiles so only 2 PV output
    tiles are ever live. No max-subtraction: scores*scale ~ N(0,1), exp is safe.
  - Partially-masked 128x128 chunks are zeroed by a bf16 multiply with
    host-derived deduped mask tiles on the vector engine. Fully-masked chunks
    are never computed; fully-allowed chunks are untouched.
  - P^T lands exactly in the layout the PV matmul needs (k on partitions):
    O[q_sub 128, 132] += P^T[:, chunk]^T @ V'[k_block] accumulated over k
    blocks in PSUM, where V' is V in bf16 with a ones column appended at col
    128 — so O[:, 128] is the softmax denominator for free. Both q_subs of a
    256-wide megatile share one PSUM bank ([128, 2, 132]).
  - reciprocal + per-partition scale normalizes, then DMA out as f32.
  - inputs stream per head (quartered, 2-head lookahead); output DMAs trigger
    from the idle gpsimd sequencer because each dma_start costs ~620ns of
    sequencer issue time and the sync queue fits only ~27 triggers per head.
"""

import os
import sys
from collections import defaultdict

import numpy as np

try:  # the repo root that provides `concourse` / `gauge`
    import concourse.bass  # noqa: F401
except ImportError:  # pragma: no cover
    for _p in ("/opt/trn_rl_repo", "/root/.axon_site/_ro/trn_rl_repo"):
        if os.path.isdir(_p) and _p not in sys.path:
            sys.path.insert(0, _p)

import ml_dtypes

B, H, S, D = 4, 16, 2048, 128
BH = B * H
NCORES = 8
HPC = BH // NCORES  # heads per core = 8
QM = 256  # q megatile width; q sub-chunks of 128 map to PV output tiles
CH = 128  # q chunk granularity (PV stationary width / mask tile width)
KB = 128  # k block (PSUM partition dim of S^T)
NM = S // QM  # 8 q megatiles
NKB = S // KB  # 16 k blocks
VW = D + 4  # V' width: col D holds ones (softmax denom), cols D+1.. are zero pad
GCH = 12  # exp bin size in chunks: [128, 1536] f32 = 3 PSUM banks
SCALE = float(np.float32(1.0 / np.sqrt(np.float32(D))))
NSUB = QM // CH  # q sub-chunks per megatile = 2

_CACHE: dict = {}
LAST_RESULTS = None  # BassKernelResults of the most recent run (for test harness)


def _derive_schedule(attn_mask):
    """Derive the chunk-level block schedule from the actual mask.

    Returns (bins, contrib, mask_tiles):
      bins: list of exp-group bins; each bin is an ordered list of chunks
        (m, j, lo, mask_id_or_None) where lo is the 128-aligned q-column
        offset within megatile m and mask_id indexes mask_tiles (None = fully
        allowed). Bins hold <= GCH chunks and never span more than 2 distinct
        megatiles (so only 2 PV accumulators are live at once).
      contrib: {(m, sub): n} count of PV contributions per output sub-tile.
      mask_tiles: [128, n_masks, CH] bf16 deduped transposed chunk masks.
    """
    am = np.asarray(attn_mask) != 0  # [S(q), S(k)]
    uniq: dict = {}
    tiles = []
    chunks = []  # (m, j, lo, mask_id|None)
    contrib: dict = defaultdict(int)
    for m in range(NM):
        for j in range(NKB):
            for c in range(NSUB):
                cm = am[m * QM + c * CH : m * QM + (c + 1) * CH, j * KB : (j + 1) * KB]
                if not cm.any():
                    continue
                if cm.all():
                    mid = None
                else:
                    key = cm.tobytes()
                    if key not in uniq:
                        uniq[key] = len(tiles)
                        tiles.append(cm.T.astype(ml_dtypes.bfloat16))  # [KB, CH]
                    mid = uniq[key]
                chunks.append((m, j, c * CH, mid))
                contrib[(m, c)] += 1

    # pack chunks into balanced bins (one ACT instruction each, sizes within
    # one chunk of each other): uniform sizes keep the PE's per-bin work
    # matched to each exp so neither engine ever waits long on the other
    def pack_even(lst):
        n = max(1, -(-len(lst) // GCH))
        base, r = divmod(len(lst), n)
        out = []
        i = 0
        for k in range(n):
            w = base + (1 if k < r else 0)
            out.append(lst[i : i + w])
            i += w
        return out

    bins = pack_even(chunks)

    # head 0 has no DMA lead time: give it megatile-aligned "ramp" bins so
    # the first exps only need the first few hundred q/k columns to land
    ramp, rest = [], list(chunks)
    for cut in (1, 2, 3):
        pre = [c for c in rest if c[0] < cut]
        if not pre or len(pre) > GCH:
            break
        ramp.append(pre)
        rest = rest[len(pre) :]
    bins0 = ramp + pack_even(rest) if ramp else bins

    # the LAST head runs megatiles in descending order so the kernel's final
    # megatile is the smallest one — its PV tail, finalize, and output DMA
    # are what the NEFF drain waits on
    desc = [c for m in reversed(range(NM)) for c in chunks if c[0] == m]
    binsL = pack_even(desc)

    mask_tiles = np.stack(tiles, axis=1) if tiles else None  # [128, n, CH]
    return bins, bins0, binsL, dict(contrib), mask_tiles


def _build_program(bins, bins0, binsL, contrib, n_masks, use_pad):
    import concourse.mybir as mybir
    import concourse.tile as tile
    from concourse import bacc

    f32 = mybir.dt.float32
    bf16 = mybir.dt.bfloat16
    Exp = mybir.ActivationFunctionType.Exp

    GCOLS = GCH * CH  # 1536

    nc = bacc.Bacc(None)
    qt_ext = nc.declare_dram_parameter("qt", [HPC, 128, S], bf16, isOutput=False)
    kt_ext = nc.declare_dram_parameter("kt", [HPC, 128, S], bf16, isOutput=False)
    vp_ext = nc.declare_dram_parameter("vp", [HPC, 128, NKB, VW], bf16, isOutput=False)
    if n_masks:
        mk_ext = nc.declare_dram_parameter("mk", [128, n_masks, CH], bf16, isOutput=False)
    if use_pad:
        pc_ext = nc.declare_dram_parameter("pc", [128, NKB], f32, isOutput=False)
    out_ext = nc.declare_dram_parameter("out", [HPC, S, D], f32, isOutput=True)

    with tile.TileContext(nc) as tc:
        with (
            tc.tile_pool(name="qt", bufs=3) as qt_pool,
            tc.tile_pool(name="kt", bufs=3) as kt_pool,
            tc.tile_pool(name="vp", bufs=3) as vp_pool,
            tc.tile_pool(name="pt", bufs=10) as pt_pool,
            tc.tile_pool(name="osb", bufs=12) as osb_pool,
            tc.tile_pool(name="rec", bufs=8) as rec_pool,
            tc.tile_pool(name="mk", bufs=1) as mk_pool,
            tc.tile_pool(name="warm", bufs=1) as warm_pool,
            tc.tile_pool(name="st", bufs=2, space="PSUM") as st_pool,
            tc.tile_pool(name="ops", bufs=2, space="PSUM") as o_pool,
        ):
            # mask / pad-column loads first — tiny, and PV needs them early
            if n_masks:
                mk = mk_pool.tile([128, n_masks, CH], bf16)
                nc.sync.dma_start(mk[:], mk_ext[:])
            if use_pad:
                pc = mk_pool.tile([128, NKB], f32)
                nc.sync.dma_start(pc[:], pc_ext[:])

            # PE warm-up: ~3us of dummy matmuls during the DMA prologue trips
            # the HAM clock gate to 2.4 GHz before the first real matmul
            warm = warm_pool.tile([128, 512], bf16, name="warm")
            nc.gpsimd.memset(warm[:], 0.0)
            wo = st_pool.tile([128, GCOLS], f32, tag="st", name="wo")
            for wi in range(4):
                nc.tensor.matmul(
                    wo[:, 0:512], lhsT=warm[:, 0:128], rhs=warm[:], start=True, stop=True
                )

            # input DMA pieces: head 0's first pieces are small so the first
            # matmul starts ~3us earlier; later heads use quarters (they have
            # a full head-period of DMA lead time via the bufs=3 lookahead)
            fine = [(0, 256), (256, 512), (512, 768), (768, 1152), (1152, 1536), (1536, 2048)]
            quarters = [(i * S // 4, (i + 1) * S // 4) for i in range(4)]
            vfine = [(0, 2), (2, 6), (6, 10), (10, 16)]
            vquarters = [(i * NKB // 4, (i + 1) * NKB // 4) for i in range(4)]

            class Head:
                def __init__(self, h):
                    self.h = h
                    self.qt = qt_pool.tile([128, S], bf16, name="qt")
                    self.kt = kt_pool.tile([128, S], bf16, name="kt")
                    self.vp = vp_pool.tile([128, NKB, VW], bf16, name="vp")
                    if h == 0:
                        # head 0's pieces are fine-grained and split across TWO
                        # sequencers (kt/vp on sync, qt on the still-idle
                        # gpsimd) — trigger issue is ~640ns serial per queue
                        for i, (a, b) in enumerate(fine):
                            nc.sync.dma_start(self.kt[:, a:b], kt_ext[h, :, a:b])
                            nc.gpsimd.dma_start(self.qt[:, a:b], qt_ext[h, :, a:b])
                            if i < len(vfine):
                                va, vb = vfine[i]
                                nc.sync.dma_start(
                                    self.vp[:, va:vb, :], vp_ext[h, :, va:vb, :]
                                )
                    else:
                        # the last head runs megatiles descending, so its
                        # high q/k columns are needed first
                        rev = h == HPC - 1
                        qs = quarters[::-1] if rev else quarters
                        vqs = vquarters[::-1] if rev else vquarters
                        for i, (a, b) in enumerate(qs):
                            nc.sync.dma_start(self.kt[:, a:b], kt_ext[h, :, a:b])
                            nc.sync.dma_start(self.qt[:, a:b], qt_ext[h, :, a:b])
                            va, vb = vqs[i]
                            nc.sync.dma_start(
                                self.vp[:, va:vb, :], vp_ext[h, :, va:vb, :]
                            )
                    self.o_tiles: dict = {}
                    self.seen: dict = defaultdict(int)
                    # PV contributions per megatile (both subs -> one bank)
                    self.total = {
                        m: contrib.get((m, 0), 0) + contrib.get((m, 1), 0)
                        for m in range(NM)
                    }

            def finalize(hd, m, o):
                # one strided reciprocal covers both sub-denominators
                rec = rec_pool.tile([128, NSUB], f32, name="rec")
                nc.vector.reciprocal(rec[:], o[:, :, D])
                last = hd.h == HPC - 1 and m == 0
                for s_ in range(NSUB):
                    osb = osb_pool.tile([128, D], f32, name="osb")
                    nc.vector.tensor_scalar_mul(
                        osb[:], o[:, s_, 0:D], rec[:, s_ : s_ + 1]
                    )
                    row0 = m * QM + s_ * CH
                    # output DMA triggers alternate between the sync and
                    # gpsimd sequencers: each dma_start costs ~620ns of
                    # sequencer issue time, and trigger-queue backlog delays
                    # the transfer whose completion frees the osb slot. The
                    # kernel's final tiles are split halves across BOTH
                    # sequencers so the drain never waits on one 2.9us DMA.
                    if last:
                        nc.sync.dma_start(
                            out_ext[hd.h, row0 : row0 + CH, 0 : D // 2],
                            osb[:, 0 : D // 2],
                        )
                        nc.gpsimd.dma_start(
                            out_ext[hd.h, row0 : row0 + CH, D // 2 : D],
                            osb[:, D // 2 : D],
                        )
                    else:
                        eng = nc.sync if (m * NSUB + s_) % 2 else nc.gpsimd
                        eng.dma_start(out_ext[hd.h, row0 : row0 + CH, :], osb[:])

            def emit_pv(hd, m, sub, j, pt, pcol):
                # ONE accumulation group per megatile: both sub-regions live
                # in one PSUM bank; the group's single start=True zeroes the
                # whole bank, so every later matmul (either region) just
                # accumulates — avoids two open groups sharing a bank, which
                # TRN2 PSUM does not support (start zeroes the full bank).
                if m not in hd.o_tiles:
                    hd.o_tiles[m] = o_pool.tile([128, NSUB, VW], f32, tag="o", name="o")
                hd.seen[m] += 1
                nc.tensor.matmul(
                    hd.o_tiles[m][:, sub, :],
                    lhsT=pt[:, pcol : pcol + CH],
                    rhs=hd.vp[:, j, :],
                    start=hd.seen[m] == 1,
                    stop=hd.seen[m] == hd.total[m],
                    skip_group_check=True,
                )
                if hd.seen[m] == hd.total[m]:
                    finalize(hd, m, hd.o_tiles.pop(m))

            def emit_scores_phase(hd, bn):
                gcols = len(bn) * CH
                st = st_pool.tile([128, GCOLS], f32, tag="st", name="st")
                # scores: coalesce consecutive chunks of the same (m, j)
                # into one matmul, splitting at PSUM 512-col banks
                p = 0
                while p < len(bn):
                    m, j, lo, _ = bn[p]
                    p2 = p + 1
                    while (
                        p2 < len(bn)
                        and bn[p2][0] == m
                        and bn[p2][1] == j
                        and bn[p2][2] == bn[p2 - 1][2] + CH
                    ):
                        p2 += 1
                    w = (p2 - p) * CH
                    off = 0
                    while off < w:
                        pcol = p * CH + off
                        wseg = min(w - off, 512 - pcol % 512)
                        nc.tensor.matmul(
                            st[:, pcol : pcol + wseg],
                            lhsT=hd.kt[:, j * KB : (j + 1) * KB],
                            rhs=hd.qt[
                                :, m * QM + lo + off : m * QM + lo + off + wseg
                            ],
                            start=True,
                            stop=True,
                        )
                        off += wseg
                    p = p2
                pt = pt_pool.tile([128, GCOLS], bf16, tag="pt", name="pt")
                nc.scalar.activation(pt[:, :gcols], st[:, :gcols], Exp, scale=SCALE)
                return pt

            def emit_pv_phase(hd, bn, pt):
                # mask/pad fixups in place, then PV per chunk
                for p, (m, j, lo, mid) in enumerate(bn):
                    pcol = p * CH
                    if mid is not None:
                        nc.vector.tensor_mul(
                            pt[:, pcol : pcol + CH],
                            pt[:, pcol : pcol + CH],
                            mk[:, mid, :],
                        )
                    if use_pad:
                        nc.vector.tensor_scalar_mul(
                            pt[:, pcol : pcol + CH],
                            pt[:, pcol : pcol + CH],
                            pc[:, j : j + 1],
                        )
                    emit_pv(hd, m, lo // CH, j, pt, pcol)

            # software-pipeline by one bin ACROSS heads: emit scores+exp of
            # bin g, then the PV batch of bin g-1 — so PV-only stretches on
            # the PE always overlap an in-flight exp and the ACT engine never
            # starves (PE idle also drops the PE clock out of its top p-state)
            heads = {}

            def head_bins(h):
                if h == 0:
                    return bins0
                if h == HPC - 1:
                    return binsL
                return bins

            flat = [(h, bn) for h in range(HPC) for bn in head_bins(h)]
            prev = None
            for h, bn in flat:
                if h not in heads:
                    heads[h] = Head(h)
                pt = emit_scores_phase(heads[h], bn)
                if prev is not None:
                    emit_pv_phase(*prev)
                prev = (heads[h], bn, pt)
            emit_pv_phase(*prev)
    nc.compile()
    return nc


def _prep_inputs(q, k, v, attn_mask, pad_mask):
    q = np.asarray(q, dtype=np.float32).reshape(BH, S, D)
    k = np.asarray(k, dtype=np.float32).reshape(BH, S, D)
    v = np.asarray(v, dtype=np.float32).reshape(BH, S, D)

    qt = np.ascontiguousarray(q.transpose(0, 2, 1)).astype(ml_dtypes.bfloat16)
    kt = np.ascontiguousarray(k.transpose(0, 2, 1)).astype(ml_dtypes.bfloat16)

    # V': [BH, 128(row within k block), NKB, VW] bf16; col D = 1.0 (denominator)
    vp = np.zeros((BH, 128, NKB, VW), dtype=ml_dtypes.bfloat16)
    vblocks = v.reshape(BH, NKB, 128, D).transpose(0, 2, 1, 3)
    vp[:, :, :, :D] = vblocks.astype(ml_dtypes.bfloat16)
    vp[:, :, :, D] = 1.0

    pad = np.asarray(pad_mask).reshape(B, S) != 0
    use_pad = not bool(pad.all())
    pcs = None
    if use_pad:
        pcs = []
        for c in range(NCORES):
            b = (c * HPC) // H
            pcs.append(
                np.ascontiguousarray(pad[b].reshape(NKB, 128).T.astype(np.float32))
            )
    return qt, kt, vp, use_pad, pcs


def kernel(q, k, v, attn_mask, pad_mask):
    global LAST_RESULTS
    from concourse.bass_utils import run_bass_kernel_spmd

    try:  # tracing needs the NTFF hook; without it BASS_TRACE=1 would crash
        import antenv.axon_hooks  # noqa: F401
    except ImportError:
        os.environ["BASS_NEVER_TRACE"] = "1"

    bins, bins0, binsL, contrib, mask_tiles = _derive_schedule(attn_mask)
    qt, kt, vp, use_pad, pcs = _prep_inputs(q, k, v, attn_mask, pad_mask)
    n_masks = 0 if mask_tiles is None else mask_tiles.shape[1]

    key = (np.asarray(attn_mask).tobytes(), use_pad)
    nc = _CACHE.get(key)
    if nc is None:
        nc = _build_program(bins, bins0, binsL, contrib, n_masks, use_pad)
        _CACHE[key] = nc

    in_maps = []
    for c in range(NCORES):
        sl = slice(c * HPC, (c + 1) * HPC)
        m = {"qt": qt[sl], "kt": kt[sl], "vp": vp[sl]}
        if n_masks:
            m["mk"] = mask_tiles
        if use_pad:
            m["pc"] = pcs[c]
        in_maps.append(m)

    res = run_bass_kernel_spmd(nc, in_maps, core_ids=list(range(NCORES)))
    LAST_RESULTS = res
    out = np.concatenate([res.results[c]["out"] for c in range(NCORES)], axis=0)
    return np.ascontiguousarray(out.reshape(B, H, S, D).astype(np.float32))


# revision 53
# speedup vs baseline: 1.0535x; 1.0535x over previous
"""Multi-head attention (B=4, H=16, S=2048, D=128, causal+pad mask) on 8 TRN2 NeuronCores.

Sharding: the 64 (batch, head) pairs are split 8 per core (pure data parallel —
attention is independent per head, no collectives needed).

Per-core kernel (per head):
  - scores are computed TRANSPOSED: S^T[k, q] = K_block^T^T @ Q^T with the
    contraction dim d=128 on partitions, the k-block (128) as the PSUM partition
    dim and the allowed q-columns (128-chunk granular, derived from the actual
    mask) as the moving dim. Q/K are host-cast to bf16.
  - The allowed 128x128 chunks are packed into PSUM group tiles of up to 12
    chunks ([128, 1536] f32 = 3 banks) so ONE scalar-engine ACTIVATE computes
    exp(scale*s) for the whole group out of PSUM into SBUF bf16 — the ACT
    engine is the critical path (1 col/cycle @1.2GHz), so group count is
    minimized. Bins never span more than 2 q-megatiles so only 2 PV output
    tiles are ever live. No max-subtraction: scores*scale ~ N(0,1), exp is safe.
  - Partially-masked 128x128 chunks are zeroed by a bf16 multiply with
    host-derived deduped mask tiles on the vector engine. Fully-masked chunks
    are never computed; fully-allowed chunks are untouched.
  - P^T lands exactly in the layout the PV matmul needs (k on partitions):
    O[q_sub 128, 132] += P^T[:, chunk]^T @ V'[k_block] accumulated over k
    blocks in PSUM, where V' is V in bf16 with a ones column appended at col
    128 — so O[:, 128] is the softmax denominator for free. Both q_subs of a
    256-wide megatile share one PSUM bank ([128, 2, 132]).
  - reciprocal + per-partition scale normalizes, then DMA out as f32.
  - inputs stream per head (quartered, 2-head lookahead); output DMAs trigger
    from the idle gpsimd sequencer because each dma_start costs ~620ns of
    sequencer issue time and the sync queue fits only ~27 triggers per head.
"""

import os
import sys
from collections import defaultdict

import numpy as np

try:  # the repo root that provides `concourse` / `gauge`
    import concourse.bass  # noqa: F401
except ImportError:  # pragma: no cover
    for _p in ("/opt/trn_rl_repo", "/root/.axon_site/_ro/trn_rl_repo"):
        if os.path.isdir(_p) and _p not in sys.path:
            sys.path.insert(0, _p)

import ml_dtypes

B, H, S, D = 4, 16, 2048, 128
BH = B * H
NCORES = 8
HPC = BH // NCORES  # heads per core = 8
QM = 256  # q megatile width; q sub-chunks of 128 map to PV output tiles
CH = 128  # q chunk granularity (PV stationary width / mask tile width)
KB = 128  # k block (PSUM partition dim of S^T)
NM = S // QM  # 8 q megatiles
NKB = S // KB  # 16 k blocks
VW = D + 4  # V' width: col D holds ones (softmax denom), cols D+1.. are zero pad
GCH = 12  # exp bin size in chunks: [128, 1536] f32 = 3 PSUM banks
SCALE = float(np.float32(1.0 / np.sqrt(np.float32(D))))
NSUB = QM // CH  # q sub-chunks per megatile = 2

_CACHE: dict = {}
LAST_RESULTS = None  # BassKernelResults of the most recent run (for test harness)


def _derive_schedule(attn_mask):
    """Derive the chunk-level block schedule from the actual mask.

    Returns (bins, contrib, mask_tiles):
      bins: list of exp-group bins; each bin is an ordered list of chunks
        (m, j, lo, mask_id_or_None) where lo is the 128-aligned q-column
        offset within megatile m and mask_id indexes mask_tiles (None = fully
        allowed). Bins hold <= GCH chunks and never span more than 2 distinct
        megatiles (so only 2 PV accumulators are live at once).
      contrib: {(m, sub): n} count of PV contributions per output sub-tile.
      mask_tiles: [128, n_masks, CH] bf16 deduped transposed chunk masks.
    """
    am = np.asarray(attn_mask) != 0  # [S(q), S(k)]
    uniq: dict = {}
    tiles = []
    chunks = []  # (m, j, lo, mask_id|None)
    contrib: dict = defaultdict(int)
    for m in range(NM):
        for j in range(NKB):
            for c in range(NSUB):
                cm = am[m * QM + c * CH : m * QM + (c + 1) * CH, j * KB : (j + 1) * KB]
                if not cm.any():
                    continue
                if cm.all():
                    mid = None
                else:
                    key = cm.tobytes()
                    if key not in uniq:
                        uniq[key] = len(tiles)
                        tiles.append(cm.T.astype(ml_dtypes.bfloat16))  # [KB, CH]
                    mid = uniq[key]
                chunks.append((m, j, c * CH, mid))
                contrib[(m, c)] += 1

    # pack chunks into balanced bins (one ACT instruction each, sizes within
    # one chunk of each other): uniform sizes keep the PE's per-bin work
    # matched to each exp so neither engine ever waits long on the other
    def pack_even(lst):
        n = max(1, -(-len(lst) // GCH))
        base, r = divmod(len(lst), n)
        out = []
        i = 0
        for k in range(n):
            w = base + (1 if k < r else 0)
            out.append(lst[i : i + w])
            i += w
        return out

    bins = pack_even(chunks)

    # head 0 has no DMA lead time: give it megatile-aligned "ramp" bins so
    # the first exps only need the first few hundred q/k columns to land
    ramp, rest = [], list(chunks)
    for cut in (1, 2, 3):
        pre = [c for c in rest if c[0] < cut]
        if not pre or len(pre) > GCH:
            break
        ramp.append(pre)
        rest = rest[len(pre) :]
    bins0 = ramp + pack_even(rest) if ramp else bins

    # the LAST head runs megatiles in descending order so the kernel's final
    # megatile is the smallest one — its PV tail, finalize, and output DMA
    # are what the NEFF drain waits on
    desc = [c for m in reversed(range(NM)) for c in chunks if c[0] == m]
    binsL = pack_even(desc)

    mask_tiles = np.stack(tiles, axis=1) if tiles else None  # [128, n, CH]
    return bins, bins0, binsL, dict(contrib), mask_tiles


def _build_program(bins, bins0, binsL, contrib, n_masks, use_pad):
    import concourse.mybir as mybir
    import concourse.tile as tile
    from concourse import bacc

    f32 = mybir.dt.float32
    bf16 = mybir.dt.bfloat16
    Exp = mybir.ActivationFunctionType.Exp

    GCOLS = GCH * CH  # 1536

    nc = bacc.Bacc(None)
    qt_ext = nc.declare_dram_parameter("qt", [HPC, 128, S], bf16, isOutput=False)
    kt_ext = nc.declare_dram_parameter("kt", [HPC, 128, S], bf16, isOutput=False)
    vp_ext = nc.declare_dram_parameter("vp", [HPC, 128, NKB, VW], bf16, isOutput=False)
    if n_masks:
        mk_ext = nc.declare_dram_parameter("mk", [128, n_masks, CH], bf16, isOutput=False)
    if use_pad:
        pc_ext = nc.declare_dram_parameter("pc", [128, NKB], f32, isOutput=False)
    out_ext = nc.declare_dram_parameter("out", [HPC, S, D], f32, isOutput=True)

    with tile.TileContext(nc) as tc:
        with (
            tc.tile_pool(name="qt", bufs=3) as qt_pool,
            tc.tile_pool(name="kt", bufs=3) as kt_pool,
            tc.tile_pool(name="vp", bufs=3) as vp_pool,
            tc.tile_pool(name="pt", bufs=10) as pt_pool,
            tc.tile_pool(name="osb", bufs=12) as osb_pool,
            tc.tile_pool(name="rec", bufs=8) as rec_pool,
            tc.tile_pool(name="mk", bufs=1) as mk_pool,
            tc.tile_pool(name="warm", bufs=1) as warm_pool,
            tc.tile_pool(name="st", bufs=2, space="PSUM") as st_pool,
            tc.tile_pool(name="ops", bufs=2, space="PSUM") as o_pool,
        ):
            # mask / pad-column loads first — tiny, and PV needs them early
            if n_masks:
                mk = mk_pool.tile([128, n_masks, CH], bf16)
                nc.sync.dma_start(mk[:], mk_ext[:])
            if use_pad:
                pc = mk_pool.tile([128, NKB], f32)
                nc.sync.dma_start(pc[:], pc_ext[:])

            # PE warm-up: ~3us of dummy matmuls during the DMA prologue trips
            # the HAM clock gate to 2.4 GHz before the first real matmul
            warm = warm_pool.tile([128, 512], bf16, name="warm")
            nc.gpsimd.memset(warm[:], 0.0)
            wo = st_pool.tile([128, GCOLS], f32, tag="st", name="wo")
            for wi in range(7):
                nc.tensor.matmul(
                    wo[:, 0:512], lhsT=warm[:, 0:128], rhs=warm[:], start=True, stop=True
                )

            # input DMA pieces: head 0's first pieces are small so the first
            # matmul starts ~3us earlier; later heads use quarters (they have
            # a full head-period of DMA lead time via the bufs=3 lookahead)
            fine = [(0, 256), (256, 512), (512, 768), (768, 1152), (1152, 1536), (1536, 2048)]
            quarters = [(i * S // 4, (i + 1) * S // 4) for i in range(4)]
            vfine = [(0, 2), (2, 6), (6, 10), (10, 16)]
            vquarters = [(i * NKB // 4, (i + 1) * NKB // 4) for i in range(4)]

            class Head:
                def __init__(self, h):
                    self.h = h
                    self.qt = qt_pool.tile([128, S], bf16, name="qt")
                    self.kt = kt_pool.tile([128, S], bf16, name="kt")
                    self.vp = vp_pool.tile([128, NKB, VW], bf16, name="vp")
                    if h == 0:
                        # head 0's pieces are fine-grained and split across TWO
                        # sequencers (kt/vp on sync, qt on the still-idle
                        # gpsimd) — trigger issue is ~640ns serial per queue
                        for i, (a, b) in enumerate(fine):
                            nc.sync.dma_start(self.kt[:, a:b], kt_ext[h, :, a:b])
                            nc.gpsimd.dma_start(self.qt[:, a:b], qt_ext[h, :, a:b])
                            if i < len(vfine):
                                va, vb = vfine[i]
                                nc.sync.dma_start(
                                    self.vp[:, va:vb, :], vp_ext[h, :, va:vb, :]
                                )
                    else:
                        for i, (a, b) in enumerate(quarters):
                            nc.sync.dma_start(self.kt[:, a:b], kt_ext[h, :, a:b])
                            nc.sync.dma_start(self.qt[:, a:b], qt_ext[h, :, a:b])
                            va, vb = vquarters[i]
                            nc.sync.dma_start(
                                self.vp[:, va:vb, :], vp_ext[h, :, va:vb, :]
                            )
                    self.o_tiles: dict = {}
                    self.seen: dict = defaultdict(int)
                    # PV contributions per megatile (both subs -> one bank)
                    self.total = {
                        m: contrib.get((m, 0), 0) + contrib.get((m, 1), 0)
                        for m in range(NM)
                    }

            def finalize(hd, m, o):
                # one strided reciprocal covers both sub-denominators
                rec = rec_pool.tile([128, NSUB], f32, name="rec")
                nc.vector.reciprocal(rec[:], o[:, :, D])
                last = hd.h == HPC - 1 and m == NM - 1
                for s_ in range(NSUB):
                    osb = osb_pool.tile([128, D], f32, name="osb")
                    nc.vector.tensor_scalar_mul(
                        osb[:], o[:, s_, 0:D], rec[:, s_ : s_ + 1]
                    )
                    row0 = m * QM + s_ * CH
                    # output DMA triggers alternate between the sync and
                    # gpsimd sequencers: each dma_start costs ~620ns of
                    # sequencer issue time, and trigger-queue backlog delays
                    # the transfer whose completion frees the osb slot. The
                    # kernel's final tiles are split halves across BOTH
                    # sequencers so the drain never waits on one 2.9us DMA.
                    if last:
                        nc.sync.dma_start(out_ext[hd.h, row0 : row0 + CH, :], osb[:])
                    else:
                        eng = nc.sync if (m * NSUB + s_) % 2 else nc.gpsimd
                        eng.dma_start(out_ext[hd.h, row0 : row0 + CH, :], osb[:])

            def emit_pv(hd, m, sub, j, pt, pcol):
                # ONE accumulation group per megatile: both sub-regions live
                # in one PSUM bank; the group's single start=True zeroes the
                # whole bank, so every later matmul (either region) just
                # accumulates — avoids two open groups sharing a bank, which
                # TRN2 PSUM does not support (start zeroes the full bank).
                if m not in hd.o_tiles:
                    hd.o_tiles[m] = o_pool.tile([128, NSUB, VW], f32, tag="o", name="o")
                hd.seen[m] += 1
                nc.tensor.matmul(
                    hd.o_tiles[m][:, sub, :],
                    lhsT=pt[:, pcol : pcol + CH],
                    rhs=hd.vp[:, j, :],
                    start=hd.seen[m] == 1,
                    stop=hd.seen[m] == hd.total[m],
                    skip_group_check=True,
                )
                if hd.seen[m] == hd.total[m]:
                    finalize(hd, m, hd.o_tiles.pop(m))

            def emit_scores_phase(hd, bn):
                gcols = len(bn) * CH
                st = st_pool.tile([128, GCOLS], f32, tag="st", name="st")
                # scores: coalesce consecutive chunks of the same (m, j)
                # into one matmul, splitting at PSUM 512-col banks
                p = 0
                while p < len(bn):
                    m, j, lo, _ = bn[p]
                    p2 = p + 1
                    while (
                        p2 < len(bn)
                        and bn[p2][0] == m
                        and bn[p2][1] == j
                        and bn[p2][2] == bn[p2 - 1][2] + CH
                    ):
                        p2 += 1
                    w = (p2 - p) * CH
                    off = 0
                    while off < w:
                        pcol = p * CH + off
                        wseg = min(w - off, 512 - pcol % 512)
                        nc.tensor.matmul(
                            st[:, pcol : pcol + wseg],
                            lhsT=hd.kt[:, j * KB : (j + 1) * KB],
                            rhs=hd.qt[
                                :, m * QM + lo + off : m * QM + lo + off + wseg
                            ],
                            start=True,
                            stop=True,
                        )
                        off += wseg
                    p = p2
                pt = pt_pool.tile([128, GCOLS], bf16, tag="pt", name="pt")
                nc.scalar.activation(pt[:, :gcols], st[:, :gcols], Exp, scale=SCALE)
                return pt

            def emit_pv_phase(hd, bn, pt):
                # mask/pad fixups in place, then PV per chunk
                for p, (m, j, lo, mid) in enumerate(bn):
                    pcol = p * CH
                    if mid is not None:
                        nc.vector.tensor_mul(
                            pt[:, pcol : pcol + CH],
                            pt[:, pcol : pcol + CH],
                            mk[:, mid, :],
                        )
                    if use_pad:
                        nc.vector.tensor_scalar_mul(
                            pt[:, pcol : pcol + CH],
                            pt[:, pcol : pcol + CH],
                            pc[:, j : j + 1],
                        )
                    emit_pv(hd, m, lo // CH, j, pt, pcol)

            # software-pipeline by one bin ACROSS heads: emit scores+exp of
            # bin g, then the PV batch of bin g-1 — so PV-only stretches on
            # the PE always overlap an in-flight exp and the ACT engine never
            # starves (PE idle also drops the PE clock out of its top p-state)
            heads = {}

            def head_bins(h):
                return bins0 if h == 0 else bins

            flat = [(h, bn) for h in range(HPC) for bn in head_bins(h)]
            prev = None
            for h, bn in flat:
                if h not in heads:
                    heads[h] = Head(h)
                pt = emit_scores_phase(heads[h], bn)
                if prev is not None:
                    emit_pv_phase(*prev)
                prev = (heads[h], bn, pt)
            emit_pv_phase(*prev)
    nc.compile()
    return nc


def _prep_inputs(q, k, v, attn_mask, pad_mask):
    q = np.asarray(q, dtype=np.float32).reshape(BH, S, D)
    k = np.asarray(k, dtype=np.float32).reshape(BH, S, D)
    v = np.asarray(v, dtype=np.float32).reshape(BH, S, D)

    qt = np.ascontiguousarray(q.transpose(0, 2, 1)).astype(ml_dtypes.bfloat16)
    kt = np.ascontiguousarray(k.transpose(0, 2, 1)).astype(ml_dtypes.bfloat16)

    # V': [BH, 128(row within k block), NKB, VW] bf16; col D = 1.0 (denominator)
    vp = np.zeros((BH, 128, NKB, VW), dtype=ml_dtypes.bfloat16)
    vblocks = v.reshape(BH, NKB, 128, D).transpose(0, 2, 1, 3)
    vp[:, :, :, :D] = vblocks.astype(ml_dtypes.bfloat16)
    vp[:, :, :, D] = 1.0

    pad = np.asarray(pad_mask).reshape(B, S) != 0
    use_pad = not bool(pad.all())
    pcs = None
    if use_pad:
        pcs = []
        for c in range(NCORES):
            b = (c * HPC) // H
            pcs.append(
                np.ascontiguousarray(pad[b].reshape(NKB, 128).T.astype(np.float32))
            )
    return qt, kt, vp, use_pad, pcs


def kernel(q, k, v, attn_mask, pad_mask):
    global LAST_RESULTS
    from concourse.bass_utils import run_bass_kernel_spmd

    try:  # tracing needs the NTFF hook; without it BASS_TRACE=1 would crash
        import antenv.axon_hooks  # noqa: F401
    except ImportError:
        os.environ["BASS_NEVER_TRACE"] = "1"

    bins, bins0, binsL, contrib, mask_tiles = _derive_schedule(attn_mask)
    qt, kt, vp, use_pad, pcs = _prep_inputs(q, k, v, attn_mask, pad_mask)
    n_masks = 0 if mask_tiles is None else mask_tiles.shape[1]

    key = (np.asarray(attn_mask).tobytes(), use_pad)
    nc = _CACHE.get(key)
    if nc is None:
        nc = _build_program(bins, bins0, binsL, contrib, n_masks, use_pad)
        _CACHE[key] = nc

    in_maps = []
    for c in range(NCORES):
        sl = slice(c * HPC, (c + 1) * HPC)
        m = {"qt": qt[sl], "kt": kt[sl], "vp": vp[sl]}
        if n_masks:
            m["mk"] = mask_tiles
        if use_pad:
            m["pc"] = pcs[c]
        in_maps.append(m)

    res = run_bass_kernel_spmd(nc, in_maps, core_ids=list(range(NCORES)))
    LAST_RESULTS = res
    out = np.concatenate([res.results[c]["out"] for c in range(NCORES)], axis=0)
    return np.ascontiguousarray(out.reshape(B, H, S, D).astype(np.float32))
